# revision 6
# baseline (speedup 1.0000x reference)
"""Trainium2 Bass kernel for nn_ConvG (3-level GCN + TopK pooling + readout).

Strategy: data-parallel over the 8 NeuronCores (16 graphs each). On the host,
the edge list is converted to dense per-graph adjacency count matrices
A[g][s][d] = #edges(s->d) + I (a pure format conversion; the initial edge
mask is all-ones so this is data-independent). On device, everything runs in
the original 256-node index space with a cumulative keep-mask kv per node:

  prop:   deg[d] = sum_s kv[s] * A[s,d]  (= 1 + masked in-degree for kept d)
          dinv = 1/sqrt(deg);  2 hops of h <- (dinv*kv) o (A^T @ ((dinv*kv) o h))
          with an intermediate (dinv^2*kv) scale between hops
  pool:   score = h @ (pw/||pw||); top-k among active rows via max8/match-
          replace; h <- h * kv_new * tanh(score)
  readout: masked max (additive -1e30 at dropped nodes) and sum/k over nodes

This reproduces the reference exactly up to fp32 rounding (validated against
the JAX reference in numpy at ~1e-5 rel err).
"""
import numpy as np

G = 16            # graphs per core
N = 256           # nodes per graph
F_IN = 128
H1 = 256
H2 = 128
C = 10
NCORES = 8
B = G * NCORES    # 128 graphs
KS = [205, 164, 132]
BIG = 1e30
MINV = -1e30

_CACHE = {}


def _build():
    import concourse.bacc as bacc
    import concourse.mybir as mybir
    import concourse.tile as tile
    from concourse.masks import make_identity

    dt = mybir.dt.float32
    AF = mybir.ActivationFunctionType
    OP = mybir.AluOpType
    AX = mybir.AxisListType

    nc = bacc.Bacc("TRN2", target_bir_lowering=False, debug=False)

    x_d = nc.dram_tensor("x", [G * N, F_IN], dt, kind="ExternalInput")
    adj_d = nc.dram_tensor("adj", [G, N, N], dt, kind="ExternalInput")
    w12_d = nc.dram_tensor("w12", [F_IN, H1], dt, kind="ExternalInput")
    w22_d = nc.dram_tensor("w22", [H1, H1], dt, kind="ExternalInput")
    w32_d = nc.dram_tensor("w32", [H1, H1], dt, kind="ExternalInput")
    w1_d = nc.dram_tensor("w1", [2 * H1, H1], dt, kind="ExternalInput")
    w2_d = nc.dram_tensor("w2", [H1, H2], dt, kind="ExternalInput")
    w3_d = nc.dram_tensor("w3", [H2, C], dt, kind="ExternalInput")
    b12_d = nc.dram_tensor("b12", [1, H1], dt, kind="ExternalInput")
    b22_d = nc.dram_tensor("b22", [1, H1], dt, kind="ExternalInput")
    b32_d = nc.dram_tensor("b32", [1, H1], dt, kind="ExternalInput")
    b1_d = nc.dram_tensor("b1", [1, H1], dt, kind="ExternalInput")
    b2_d = nc.dram_tensor("b2", [1, H2], dt, kind="ExternalInput")
    b3_d = nc.dram_tensor("b3", [1, C], dt, kind="ExternalInput")
    pwb_d = [nc.dram_tensor(f"pwb{i}", [128, H1], dt, kind="ExternalInput")
             for i in range(3)]
    out_d = nc.dram_tensor("out", [G, C], dt, kind="ExternalOutput")

    GN = G * N  # 4096

    with tile.TileContext(nc) as tc:
        import contextlib
        with contextlib.ExitStack() as ctx:
            big = ctx.enter_context(tc.tile_pool(name="big", bufs=1))
            sm = ctx.enter_context(tc.tile_pool(name="sm", bufs=1))
            sq_pool = ctx.enter_context(tc.tile_pool(name="sqp", bufs=3))
            pmm = ctx.enter_context(tc.tile_pool(name="pmm", bufs=3, space="PSUM"))
            pt = ctx.enter_context(tc.tile_pool(name="pt", bufs=3, space="PSUM"))
            pdeg = ctx.enter_context(tc.tile_pool(name="pdeg", bufs=2, space="PSUM"))

            HN = big.tile([128, 2 * GN], dt, tag="HN")    # node-major h
            U = big.tile([128, 2 * GN], dt, tag="U")      # scratch
            HT = [big.tile([128, GN], dt, tag=f"HT{i}", name=f"HT{i}") for i in range(2)]
            ADJ = big.tile([128, 2 * GN], dt, tag="ADJ")

            W12S = sm.tile([128, H1], dt, tag="w12s")
            W22S = sm.tile([128, 2 * H1], dt, tag="w22s")
            W32S = sm.tile([128, 2 * H1], dt, tag="w32s")
            W1S = sm.tile([128, 4 * H1], dt, tag="w1s")
            W2S = sm.tile([128, 2 * H2], dt, tag="w2s")
            W3S = sm.tile([128, C], dt, tag="w3s")
            B12R = sm.tile([1, H1], dt, tag="b12r")
            B22R = sm.tile([1, H1], dt, tag="b22r")
            B32R = sm.tile([1, H1], dt, tag="b32r")
            B1R = sm.tile([1, H1], dt, tag="b1r")
            B2R = sm.tile([1, H2], dt, tag="b2r")
            B3R = sm.tile([1, C], dt, tag="b3r")
            PWB = [sm.tile([128, H1], dt, tag=f"pwb{i}", name=f"PWB{i}") for i in range(3)]

            IDT = sm.tile([128, 128], dt, tag="idt")
            ONESR = sm.tile([1, 128], dt, tag="onesr")
            EPSB = sm.tile([128, 1], dt, tag="epsb")

            KV = sm.tile([16, N], dt, tag="kv")
            KVT = [sm.tile([128, G], dt, tag=f"kvt{i}", name=f"KVT{i}") for i in range(2)]
            DIC = [sm.tile([128, G], dt, tag=f"dic{i}", name=f"DIC{i}") for i in range(2)]
            KD = [sm.tile([128, G], dt, tag=f"kd{i}", name=f"KD{i}") for i in range(2)]
            KD2 = [sm.tile([128, G], dt, tag=f"kd2{i}", name=f"KD2{i}") for i in range(2)]
            SCC = [sm.tile([128, G], dt, tag=f"scc{i}", name=f"SCC{i}") for i in range(2)]
            PST = [sm.tile([128, G], dt, tag=f"pst{i}", name=f"PST{i}") for i in range(2)]
            SC = sm.tile([16, N], dt, tag="sc")
            SCM = sm.tile([16, N], dt, tag="scm")
            AM16 = sm.tile([16, N], dt, tag="am16")
            WRK = sm.tile([16, N], dt, tag="wrk")
            MSK = sm.tile([16, N], dt, tag="msk")
            TH = sm.tile([16, N], dt, tag="th")
            PS = sm.tile([16, N], dt, tag="ps")
            TK8 = sm.tile([16, 8], dt, tag="tk8")

            ZACC = sm.tile([128, 64], dt, tag="zacc")
            ZTL = sm.tile([128, 64], dt, tag="ztl")
            Z1 = sm.tile([16, H1], dt, tag="z1")
            Z1T = sm.tile([128, 2 * G], dt, tag="z1t")
            Z2 = sm.tile([16, H2], dt, tag="z2")
            Z2T = sm.tile([128, G], dt, tag="z2t")
            M16 = sm.tile([16, 1], dt, tag="m16")
            NM16 = sm.tile([16, 1], dt, tag="nm16")
            ES = sm.tile([16, C], dt, tag="es")
            SE = sm.tile([16, 1], dt, tag="se")
            LSE = sm.tile([16, 1], dt, tag="lse")
            OUTS = sm.tile([16, C], dt, tag="outs")

            def hsl(g, t):  # HN/U/T1 slice for (graph, node-half)
                o = (g * 2 + t) * N
                return slice(o, o + N)

            # ---- consts + input DMAs
            make_identity(nc, IDT[:])
            nc.gpsimd.memset(ONESR[:], 1.0)
            nc.gpsimd.memset(EPSB[:], 1e-12)
            nc.gpsimd.memset(KV[:], 1.0)
            nc.gpsimd.memset(KVT[0][:], 1.0)
            nc.gpsimd.memset(KVT[1][:], 1.0)
            nc.gpsimd.memset(ZACC[:], 0.0)

            nc.sync.dma_start(U[:, 0:GN].rearrange("p (t f) -> p t f", t=32),
                  x_d[:].rearrange("(t p) f -> p t f", p=128))
            nc.sync.dma_start(ADJ[:].rearrange("p (g t d) -> p g t d", g=G, t=2),
                  adj_d[:].rearrange("g (t p) d -> p g t d", p=128))
            nc.sync.dma_start(W12S[:], w12_d[:])
            nc.sync.dma_start(W22S[:].rearrange("p (t n) -> p t n", n={"W22S":256,"W32S":256,"W1S":256,"W2S":128}["W22S"]),
                  w22_d[:].rearrange("(t p) n -> p t n", p=128))
            nc.sync.dma_start(W32S[:].rearrange("p (t n) -> p t n", n={"W22S":256,"W32S":256,"W1S":256,"W2S":128}["W32S"]),
                  w32_d[:].rearrange("(t p) n -> p t n", p=128))
            nc.sync.dma_start(W1S[:].rearrange("p (t n) -> p t n", n={"W22S":256,"W32S":256,"W1S":256,"W2S":128}["W1S"]),
                  w1_d[:].rearrange("(t p) n -> p t n", p=128))
            nc.sync.dma_start(W2S[:].rearrange("p (t n) -> p t n", n={"W22S":256,"W32S":256,"W1S":256,"W2S":128}["W2S"]),
                  w2_d[:].rearrange("(t p) n -> p t n", p=128))
            nc.sync.dma_start(W3S[:], w3_d[:])
            for dst, src in ((B12R, b12_d), (B22R, b22_d), (B32R, b32_d),
                             (B1R, b1_d), (B2R, b2_d), (B3R, b3_d)):
                nc.sync.dma_start(dst[:], src[:])
            for i in range(3):
                nc.sync.dma_start(PWB[i][:], pwb_d[i][:])

            # ---- xT into HT0 (level-1 feature-major input; F_IN = 128)
            for i in range(32):
                pp = pt.tile([128, 128], dt, tag="psT")
                nc.tensor.transpose(pp[:], U[:, i * 128:(i + 1) * 128], IDT[:])
                if i % 2 == 0:
                    nc.scalar.copy(HT[0][:, i * 128:(i + 1) * 128], pp[:])
                else:
                    nc.vector.tensor_copy(HT[0][:, i * 128:(i + 1) * 128], pp[:])

            def dense(lvl):
                """HT (feature-major) -> HN = relu(h @ W + b), node-major."""
                WS, BR, kts = {1: (W12S, B12R, 1), 2: (W22S, B22R, 2),
                               3: (W32S, B32R, 2)}[lvl]
                for g in range(G):
                    for mt in range(2):
                        ps = pmm.tile([128, H1], dt, tag="psA")
                        o = g * N + mt * 128
                        for kt in range(kts):
                            nc.tensor.matmul(
                                ps[:], HT[kt][:, o:o + 128],
                                WS[:, kt * H1:(kt + 1) * H1],
                                start=(kt == 0), stop=False)
                        nc.tensor.matmul(ps[:], ONESR[0:1, 0:128], BR[:],
                                         start=False, stop=True)
                        nc.scalar.activation(HN[:, hsl(g, mt)], ps[:], AF.Relu)

            def prop():
                # degrees -> dinv columns -> kd = dinv*kv, kd2 = dinv^2*kv
                for g in range(G):
                    for dh in range(2):
                        pc = pdeg.tile([128, 1], dt, tag="psD")
                        for st in range(2):
                            ao = (g * 2 + st) * N + dh * 128
                            nc.tensor.matmul(pc[:], ADJ[:, ao:ao + 128],
                                             KVT[st][:, g:g + 1],
                                             start=(st == 0), stop=(st == 1))
                        sqc = sq_pool.tile([128, 1], dt, tag="sq")
                        nc.scalar.activation(sqc[:], pc[:], AF.Sqrt,
                                             bias=EPSB[:, 0:1])
                        nc.vector.reciprocal(DIC[dh][:, g:g + 1], sqc[:])
                        nc.vector.tensor_mul(KD[dh][:, g:g + 1],
                                             DIC[dh][:, g:g + 1],
                                             KVT[dh][:, g:g + 1])
                        nc.vector.tensor_mul(KD2[dh][:, g:g + 1],
                                             KD[dh][:, g:g + 1],
                                             DIC[dh][:, g:g + 1])
                # u = kd o h
                for g in range(G):
                    for t in range(2):
                        nc.vector.tensor_scalar_mul(U[:, hsl(g, t)],
                                                    HN[:, hsl(g, t)],
                                                    KD[t][:, g:g + 1])
                # hop 1: u <- kd2 o (A^T @ u)   (in place, via two psums)
                for g in range(G):
                    pss = []
                    for dh in range(2):
                        ps = pmm.tile([128, H1], dt, tag="psA")
                        for st in range(2):
                            ao = (g * 2 + st) * N + dh * 128
                            nc.tensor.matmul(ps[:], ADJ[:, ao:ao + 128],
                                             U[:, hsl(g, st)],
                                             start=(st == 0), stop=(st == 1))
                        pss.append(ps)
                    for dh in range(2):
                        nc.vector.tensor_scalar_mul(U[:, hsl(g, dh)], pss[dh][:],
                                                    KD2[dh][:, g:g + 1])
                # hop 2: h = kd o (A^T @ u)
                for g in range(G):
                    for dh in range(2):
                        ps = pmm.tile([128, H1], dt, tag="psA")
                        for st in range(2):
                            ao = (g * 2 + st) * N + dh * 128
                            nc.tensor.matmul(ps[:], ADJ[:, ao:ao + 128],
                                             U[:, hsl(g, st)],
                                             start=(st == 0), stop=(st == 1))
                        nc.vector.tensor_scalar_mul(HN[:, hsl(g, dh)], ps[:],
                                                    KD[dh][:, g:g + 1])

            def pool_readout(lvl):
                k = KS[lvl]
                # scores (columns), via fused mul+reduce on DVE
                for g in range(G):
                    for mt in range(2):
                        nc.vector.tensor_mul(U[:, hsl(g, mt)],
                                             HN[:, hsl(g, mt)], PWB[lvl][:])
                        nc.vector.tensor_reduce(SCC[mt][:, g:g + 1],
                                                U[:, hsl(g, mt)],
                                                axis=AX.X, op=OP.add)
                # score rows [16, 256]
                for mt in range(2):
                    pp = pt.tile([128, 128], dt, tag="psT")
                    nc.tensor.transpose(pp[0:16, :], SCC[mt][:], IDT[:])
                    nc.scalar.copy(SC[:, mt * 128:(mt + 1) * 128], pp[0:16, :])
                # mask inactive scores to -BIG
                nc.vector.tensor_scalar(AM16[:], KV[:], 1.0, BIG,
                                        op0=OP.subtract, op1=OP.mult)
                nc.vector.tensor_add(SCM[:], SC[:], AM16[:])
                # top-k mask via max8 + match_replace
                cur = SCM
                for it in range((k + 7) // 8):
                    nc.vector.max(TK8[:], cur[:])
                    rem = k - it * 8
                    if rem < 8:
                        nc.vector.memset(TK8[:, rem:8], MINV)
                    nc.vector.match_replace(WRK[:], TK8[:], cur[:], MINV)
                    cur = WRK
                nc.vector.tensor_sub(MSK[:], SCM[:], WRK[:])
                nc.vector.tensor_scalar_min(MSK[:], MSK[:], 1.0)
                # update keep state; pool scale ps = kv * tanh(score)
                nc.scalar.activation(TH[:], SC[:], AF.Tanh)
                nc.vector.tensor_mul(KV[:], KV[:], MSK[:])
                nc.vector.tensor_mul(PS[:], KV[:], TH[:])
                for mt in range(2):
                    for src, dsts in ((PS, PST), (KV, KVT)):
                        pp = pt.tile([128, 128], dt, tag="psT")
                        nc.tensor.transpose(pp[:, 0:16],
                                            src[0:16, mt * 128:(mt + 1) * 128],
                                            IDT[0:16, 0:16])
                        nc.vector.tensor_copy(dsts[mt][:], pp[:, 0:16])
                # h <- h * ps  (zeroes dropped rows, scales kept by tanh)
                for g in range(G):
                    for t in range(2):
                        nc.vector.tensor_scalar_mul(HN[:, hsl(g, t)],
                                                    HN[:, hsl(g, t)],
                                                    PST[t][:, g:g + 1])
                # transpose to feature-major HT
                i = 0
                for g in range(G):
                    for mt in range(2):
                        for ft in range(2):
                            pp = pt.tile([128, 128], dt, tag="psT")
                            o = (g * 2 + mt) * N + ft * 128
                            nc.tensor.transpose(pp[:], HN[:, o:o + 128], IDT[:])
                            dst = HT[ft][:, g * N + mt * 128:
                                          g * N + mt * 128 + 128]
                            if i % 2 == 0:
                                nc.scalar.copy(dst, pp[:])
                            else:
                                nc.vector.tensor_copy(dst, pp[:])
                            i += 1
                # readout: additive mask AMB = (kv-1)*BIG broadcast over parts
                nc.sync.dma_start(U[0:1, 0:GN], KV[:])
                for c in range(8):
                    pb = pmm.tile([128, 512], dt, tag="psA")
                    nc.tensor.matmul(pb[:], ONESR[0:1, :],
                                     U[0:1, c * 512:(c + 1) * 512],
                                     start=True, stop=True)
                    nc.vector.tensor_scalar(U[:, GN + c * 512:GN + (c + 1) * 512],
                                            pb[:], 1.0, BIG, op0=OP.subtract,
                                            op1=OP.mult)
                for ft in range(2):
                    nc.vector.tensor_add(U[:, 0:GN], HT[ft][:], U[:, GN:2 * GN])
                    nc.vector.tensor_reduce(
                        ZTL[:, ft * 16:(ft + 1) * 16],
                        U[:, 0:GN].rearrange("p (g n) -> p g n", g=G),
                        axis=AX.X, op=OP.max)
                    nc.vector.tensor_reduce(
                        ZTL[:, (2 + ft) * 16:(3 + ft) * 16],
                        HT[ft][:].rearrange("p (g n) -> p g n", g=G),
                        axis=AX.X, op=OP.add)
                nc.vector.tensor_scalar_mul(ZTL[:, 32:64], ZTL[:, 32:64],
                                            1.0 / k)
                nc.vector.tensor_add(ZACC[:], ZACC[:], ZTL[:])

            # ---- the network
            dense(1)
            for lvl in range(3):
                prop()
                pool_readout(lvl)
                if lvl < 2:
                    dense(lvl + 2)

            # ---- final MLP + log_softmax
            ps1 = pmm.tile([128, H1], dt, tag="psA")
            for kt in range(4):
                nc.tensor.matmul(ps1[0:16, :], ZACC[:, kt * 16:(kt + 1) * 16],
                                 W1S[:, kt * H1:(kt + 1) * H1],
                                 start=(kt == 0), stop=False)
            nc.tensor.matmul(ps1[0:16, :], ONESR[0:1, 0:16], B1R[:],
                             start=False, stop=True)
            nc.scalar.activation(Z1[:], ps1[0:16, :], AF.Relu)
            for kt in range(2):
                pp = pt.tile([128, 128], dt, tag="psT")
                nc.tensor.transpose(pp[:, 0:16],
                                    Z1[0:16, kt * 128:(kt + 1) * 128],
                                    IDT[0:16, 0:16])
                nc.scalar.copy(Z1T[:, kt * 16:(kt + 1) * 16], pp[:, 0:16])
            ps2 = pmm.tile([128, H2], dt, tag="psA")
            for kt in range(2):
                nc.tensor.matmul(ps2[0:16, :], Z1T[:, kt * 16:(kt + 1) * 16],
                                 W2S[:, kt * H2:(kt + 1) * H2],
                                 start=(kt == 0), stop=False)
            nc.tensor.matmul(ps2[0:16, :], ONESR[0:1, 0:16], B2R[:],
                             start=False, stop=True)
            nc.scalar.activation(Z2[:], ps2[0:16, :], AF.Relu)
            pp = pt.tile([128, 128], dt, tag="psT")
            nc.tensor.transpose(pp[:, 0:16], Z2[0:16, :], IDT[0:16, 0:16])
            nc.scalar.copy(Z2T[:], pp[:, 0:16])
            ps3 = pmm.tile([128, C], dt, tag="psA")
            nc.tensor.matmul(ps3[0:16, :], Z2T[:], W3S[:], start=True,
                             stop=False)
            nc.tensor.matmul(ps3[0:16, :], ONESR[0:1, 0:16], B3R[:],
                             start=False, stop=True)
            nc.vector.tensor_reduce(M16[:], ps3[0:16, :], axis=AX.X, op=OP.max)
            nc.vector.tensor_scalar_mul(NM16[:], M16[:], -1.0)
            nc.scalar.activation(ES[:], ps3[0:16, :], AF.Exp,
                                 bias=NM16[0:16, 0:1], scale=1.0)
            nc.vector.tensor_reduce(SE[:], ES[:], axis=AX.X, op=OP.add)
            nc.scalar.activation(LSE[:], SE[:], AF.Ln)
            nc.vector.tensor_scalar(OUTS[:], ps3[0:16, :], M16[0:16, 0:1],
                                    LSE[0:16, 0:1], op0=OP.subtract,
                                    op1=OP.subtract)
            nc.sync.dma_start(out_d[:], OUTS[:])

    nc.compile()
    return nc


def _get_nc():
    if "nc" not in _CACHE:
        _CACHE["nc"] = _build()
    return _CACHE["nc"]


def _host_prep(inputs):
    """Build per-core input maps (shared weights + per-core x/adj slices)."""
    x = np.ascontiguousarray(np.asarray(inputs["x"], np.float32))
    edges = np.asarray(inputs["edges"], np.int32)
    # dense adjacency counts + self loop: A[g, s, d] = #edges(s->d) + I
    src = edges[..., 0].astype(np.int64)
    dst = edges[..., 1].astype(np.int64)
    gidx = np.arange(B, dtype=np.int64)[:, None]
    flat = (gidx * N * N + src * N + dst).ravel()
    A = np.bincount(flat, minlength=B * N * N).astype(np.float32)
    A = A.reshape(B, N, N)
    A += np.eye(N, dtype=np.float32)[None]

    shared = {}
    for name, key in (("w12", "W12"), ("w22", "W22"), ("w32", "W32"),
                      ("w1", "W1"), ("w2", "W2"), ("w3", "W3")):
        shared[name] = np.ascontiguousarray(np.asarray(inputs[key], np.float32))
    for name, key, n in (("b12", "b12", H1), ("b22", "b22", H1),
                         ("b32", "b32", H1), ("b1", "b1", H1),
                         ("b2", "b2", H2), ("b3", "b3", C)):
        shared[name] = np.asarray(inputs[key], np.float32).reshape(1, n)
    for i, key in enumerate(("pw1", "pw2", "pw3")):
        pw = np.asarray(inputs[key], np.float32)
        pwn = pw / np.linalg.norm(pw)
        shared[f"pwb{i}"] = np.ascontiguousarray(
            np.broadcast_to(pwn[None, :], (128, H1)).astype(np.float32))

    in_maps = []
    for c in range(NCORES):
        m = dict(shared)
        m["x"] = np.ascontiguousarray(x[c * G * N:(c + 1) * G * N])
        m["adj"] = np.ascontiguousarray(A[c * G:(c + 1) * G])
        in_maps.append(m)
    return in_maps


def kernel(**inputs):
    from concourse.bass_utils import run_bass_kernel_spmd
    nc = _get_nc()
    in_maps = _host_prep(inputs)
    r = run_bass_kernel_spmd(nc, in_maps, core_ids=list(range(NCORES)))
    out = np.concatenate([r.results[c]["out"] for c in range(NCORES)], axis=0)
    return out.astype(np.float32)


def run_traced(inputs):
    """Like kernel() but with NTFF tracing; returns (out, BassKernelResults)."""
    import sys
    import types
    if "antenv.axon_hooks" not in sys.modules:
        hooks = types.ModuleType("antenv.axon_hooks")
        hooks._hook = None
        hooks.set_axon_ntff_profile_hook = lambda h: setattr(hooks, "_hook", h)
        hooks.get_axon_ntff_profile_hook = lambda: hooks._hook
        sys.modules["antenv.axon_hooks"] = hooks
        from trn_agent_boot.trn_boot import _ntff_profile_via_ctypes
        hooks.set_axon_ntff_profile_hook(
            _ntff_profile_via_ctypes("/opt/axon/libaxon_pjrt.so"))
    from concourse.bass_utils import run_bass_kernel_spmd
    nc = _get_nc()
    in_maps = _host_prep(inputs)
    r = run_bass_kernel_spmd(nc, in_maps, core_ids=list(range(NCORES)),
                             trace=True)
    out = np.concatenate([r.results[c]["out"] for c in range(NCORES)], axis=0)
    return out.astype(np.float32), r


# revision 7
# speedup vs baseline: 1.0227x; 1.0227x over previous
"""Trainium2 Bass kernel for nn_ConvG (3-level GCN + TopK pooling + readout).

Strategy: data-parallel over the 8 NeuronCores (16 graphs each). On the host,
the edge list is converted to dense per-graph adjacency count matrices
A[g][s][d] = #edges(s->d) + I (a pure format conversion; the initial edge
mask is all-ones so this is data-independent). On device, everything runs in
the original 256-node index space with a cumulative keep-mask kv per node:

  prop:   deg[d] = sum_s kv[s] * A[s,d]  (= 1 + masked in-degree for kept d)
          dinv = 1/sqrt(deg);  2 hops of h <- (dinv*kv) o (A^T @ ((dinv*kv) o h))
          with an intermediate (dinv^2*kv) scale between hops
  pool:   score = h @ (pw/||pw||); top-k among active rows via max8/match-
          replace; h <- h * kv_new * tanh(score)
  readout: masked max (additive -1e30 at dropped nodes) and sum/k over nodes

This reproduces the reference exactly up to fp32 rounding (validated against
the JAX reference in numpy at ~1e-5 rel err).
"""
import numpy as np

G = 16            # graphs per core
N = 256           # nodes per graph
F_IN = 128
H1 = 256
H2 = 128
C = 10
NCORES = 8
B = G * NCORES    # 128 graphs
KS = [205, 164, 132]
BIG = 1e30
MINV = -1e30

_CACHE = {}


def _build():
    import concourse.bacc as bacc
    import concourse.mybir as mybir
    import concourse.tile as tile
    from concourse.masks import make_identity

    dt = mybir.dt.float32
    AF = mybir.ActivationFunctionType
    OP = mybir.AluOpType
    AX = mybir.AxisListType

    nc = bacc.Bacc("TRN2", target_bir_lowering=False, debug=False)

    x_d = nc.dram_tensor("x", [G * N, F_IN], dt, kind="ExternalInput")
    adj_d = nc.dram_tensor("adj", [G, N, N], dt, kind="ExternalInput")
    w12_d = nc.dram_tensor("w12", [F_IN, H1], dt, kind="ExternalInput")
    w22_d = nc.dram_tensor("w22", [H1, H1], dt, kind="ExternalInput")
    w32_d = nc.dram_tensor("w32", [H1, H1], dt, kind="ExternalInput")
    w1_d = nc.dram_tensor("w1", [2 * H1, H1], dt, kind="ExternalInput")
    w2_d = nc.dram_tensor("w2", [H1, H2], dt, kind="ExternalInput")
    w3_d = nc.dram_tensor("w3", [H2, C], dt, kind="ExternalInput")
    b12_d = nc.dram_tensor("b12", [1, H1], dt, kind="ExternalInput")
    b22_d = nc.dram_tensor("b22", [1, H1], dt, kind="ExternalInput")
    b32_d = nc.dram_tensor("b32", [1, H1], dt, kind="ExternalInput")
    b1_d = nc.dram_tensor("b1", [1, H1], dt, kind="ExternalInput")
    b2_d = nc.dram_tensor("b2", [1, H2], dt, kind="ExternalInput")
    b3_d = nc.dram_tensor("b3", [1, C], dt, kind="ExternalInput")
    pwb_d = [nc.dram_tensor(f"pwb{i}", [128, H1], dt, kind="ExternalInput")
             for i in range(3)]
    out_d = nc.dram_tensor("out", [G, C], dt, kind="ExternalOutput")

    GN = G * N  # 4096

    with tile.TileContext(nc) as tc:
        import contextlib
        with contextlib.ExitStack() as ctx:
            big = ctx.enter_context(tc.tile_pool(name="big", bufs=1))
            sm = ctx.enter_context(tc.tile_pool(name="sm", bufs=1))
            sq_pool = ctx.enter_context(tc.tile_pool(name="sqp", bufs=3))
            pmm = ctx.enter_context(tc.tile_pool(name="pmm", bufs=4, space="PSUM"))
            pt = ctx.enter_context(tc.tile_pool(name="pt", bufs=3, space="PSUM"))
            pdeg = ctx.enter_context(tc.tile_pool(name="pdeg", bufs=1, space="PSUM"))

            HN = big.tile([128, 2 * GN], dt, tag="HN")    # node-major h
            U = big.tile([128, 2 * GN], dt, tag="U")      # scratch
            HT = [big.tile([128, GN], dt, tag=f"HT{i}", name=f"HT{i}") for i in range(2)]
            ADJ = big.tile([128, 2 * GN], dt, tag="ADJ")

            W12S = sm.tile([128, H1], dt, tag="w12s")
            W22S = sm.tile([128, 2 * H1], dt, tag="w22s")
            W32S = sm.tile([128, 2 * H1], dt, tag="w32s")
            W1S = sm.tile([128, 4 * H1], dt, tag="w1s")
            W2S = sm.tile([128, 2 * H2], dt, tag="w2s")
            W3S = sm.tile([128, C], dt, tag="w3s")
            B12R = sm.tile([1, H1], dt, tag="b12r")
            B22R = sm.tile([1, H1], dt, tag="b22r")
            B32R = sm.tile([1, H1], dt, tag="b32r")
            B1R = sm.tile([1, H1], dt, tag="b1r")
            B2R = sm.tile([1, H2], dt, tag="b2r")
            B3R = sm.tile([1, C], dt, tag="b3r")
            PWB = [sm.tile([128, H1], dt, tag=f"pwb{i}", name=f"PWB{i}") for i in range(3)]

            IDT = sm.tile([128, 128], dt, tag="idt")
            ONESR = sm.tile([1, 128], dt, tag="onesr")
            EPSB = sm.tile([128, 1], dt, tag="epsb")

            KV = sm.tile([16, N], dt, tag="kv")
            KVT = [sm.tile([128, G], dt, tag=f"kvt{i}", name=f"KVT{i}") for i in range(2)]
            DICB = sm.tile([128, 2 * G], dt, tag="dicb")
            KD = [sm.tile([128, G], dt, tag=f"kd{i}", name=f"KD{i}") for i in range(2)]
            KD2 = [sm.tile([128, G], dt, tag=f"kd2{i}", name=f"KD2{i}") for i in range(2)]
            SCC = [sm.tile([128, G], dt, tag=f"scc{i}", name=f"SCC{i}") for i in range(2)]
            PST = [sm.tile([128, G], dt, tag=f"pst{i}", name=f"PST{i}") for i in range(2)]
            SC = sm.tile([16, N], dt, tag="sc")
            SCM = sm.tile([16, N], dt, tag="scm")
            AM16 = sm.tile([16, N], dt, tag="am16")
            WRK = sm.tile([16, N], dt, tag="wrk")
            MSK = sm.tile([16, N], dt, tag="msk")
            TH = sm.tile([16, N], dt, tag="th")
            PS = sm.tile([16, N], dt, tag="ps")
            TK8 = sm.tile([16, 8], dt, tag="tk8")

            ZACC = sm.tile([128, 64], dt, tag="zacc")
            ZTL = sm.tile([128, 64], dt, tag="ztl")
            Z1 = sm.tile([16, H1], dt, tag="z1")
            Z1T = sm.tile([128, 2 * G], dt, tag="z1t")
            Z2 = sm.tile([16, H2], dt, tag="z2")
            Z2T = sm.tile([128, G], dt, tag="z2t")
            M16 = sm.tile([16, 1], dt, tag="m16")
            NM16 = sm.tile([16, 1], dt, tag="nm16")
            ES = sm.tile([16, C], dt, tag="es")
            SE = sm.tile([16, 1], dt, tag="se")
            LSE = sm.tile([16, 1], dt, tag="lse")
            OUTS = sm.tile([16, C], dt, tag="outs")

            def hsl(g, t):  # HN/U/T1 slice for (graph, node-half)
                o = (g * 2 + t) * N
                return slice(o, o + N)

            # ---- consts + input DMAs
            make_identity(nc, IDT[:])
            nc.gpsimd.memset(ONESR[:], 1.0)
            nc.gpsimd.memset(EPSB[:], 1e-12)
            nc.gpsimd.memset(KV[:], 1.0)
            nc.gpsimd.memset(KVT[0][:], 1.0)
            nc.gpsimd.memset(KVT[1][:], 1.0)
            nc.gpsimd.memset(ZACC[:], 0.0)

            nc.sync.dma_start(U[:, 0:GN].rearrange("p (t f) -> p t f", t=32),
                  x_d[:].rearrange("(t p) f -> p t f", p=128))
            nc.sync.dma_start(ADJ[:].rearrange("p (g t d) -> p g t d", g=G, t=2),
                  adj_d[:].rearrange("g (t p) d -> p g t d", p=128))
            nc.sync.dma_start(W12S[:], w12_d[:])
            nc.sync.dma_start(W22S[:].rearrange("p (t n) -> p t n", n={"W22S":256,"W32S":256,"W1S":256,"W2S":128}["W22S"]),
                  w22_d[:].rearrange("(t p) n -> p t n", p=128))
            nc.sync.dma_start(W32S[:].rearrange("p (t n) -> p t n", n={"W22S":256,"W32S":256,"W1S":256,"W2S":128}["W32S"]),
                  w32_d[:].rearrange("(t p) n -> p t n", p=128))
            nc.sync.dma_start(W1S[:].rearrange("p (t n) -> p t n", n={"W22S":256,"W32S":256,"W1S":256,"W2S":128}["W1S"]),
                  w1_d[:].rearrange("(t p) n -> p t n", p=128))
            nc.sync.dma_start(W2S[:].rearrange("p (t n) -> p t n", n={"W22S":256,"W32S":256,"W1S":256,"W2S":128}["W2S"]),
                  w2_d[:].rearrange("(t p) n -> p t n", p=128))
            nc.sync.dma_start(W3S[:], w3_d[:])
            for dst, src in ((B12R, b12_d), (B22R, b22_d), (B32R, b32_d),
                             (B1R, b1_d), (B2R, b2_d), (B3R, b3_d)):
                nc.sync.dma_start(dst[:], src[:])
            for i in range(3):
                nc.sync.dma_start(PWB[i][:], pwb_d[i][:])

            # ---- xT into HT0 (level-1 feature-major input; F_IN = 128)
            for i in range(32):
                pp = pt.tile([128, 128], dt, tag="psT")
                nc.tensor.transpose(pp[:], U[:, i * 128:(i + 1) * 128], IDT[:])
                if i % 2 == 0:
                    nc.scalar.copy(HT[0][:, i * 128:(i + 1) * 128], pp[:])
                else:
                    nc.vector.tensor_copy(HT[0][:, i * 128:(i + 1) * 128], pp[:])

            def dense(lvl):
                """HT (feature-major) -> HN = relu(h @ W + b), node-major."""
                WS, BR, kts = {1: (W12S, B12R, 1), 2: (W22S, B22R, 2),
                               3: (W32S, B32R, 2)}[lvl]
                for g in range(G):
                    for mt in range(2):
                        ps = pmm.tile([128, H1], dt, tag="psA")
                        o = g * N + mt * 128
                        for kt in range(kts):
                            nc.tensor.matmul(
                                ps[:], HT[kt][:, o:o + 128],
                                WS[:, kt * H1:(kt + 1) * H1],
                                start=(kt == 0), stop=False)
                        nc.tensor.matmul(ps[:], ONESR[0:1, 0:128], BR[:],
                                         start=False, stop=True)
                        nc.scalar.activation(HN[:, hsl(g, mt)], ps[:], AF.Relu)

            def prop():
                # degrees -> dinv columns -> kd = dinv*kv, kd2 = dinv^2*kv
                pdg = pdeg.tile([128, 2 * G], dt, tag="psD")
                for g in range(G):
                    for dh in range(2):
                        col = dh * G + g
                        for st in range(2):
                            ao = (g * 2 + st) * N + dh * 128
                            nc.tensor.matmul(pdg[:, col:col + 1],
                                             ADJ[:, ao:ao + 128],
                                             KVT[st][:, g:g + 1],
                                             start=(st == 0), stop=(st == 1))
                sqc = sq_pool.tile([128, 2 * G], dt, tag="sq")
                nc.scalar.activation(sqc[:], pdg[:], AF.Sqrt, bias=EPSB[:, 0:1])
                nc.vector.reciprocal(DICB[:], sqc[:])
                for dh in range(2):
                    nc.vector.tensor_mul(KD[dh][:], DICB[:, dh * G:(dh + 1) * G],
                                         KVT[dh][:])
                    nc.vector.tensor_mul(KD2[dh][:], KD[dh][:],
                                         DICB[:, dh * G:(dh + 1) * G])
                # u = kd o h
                for g in range(G):
                    for t in range(2):
                        nc.vector.tensor_scalar_mul(U[:, hsl(g, t)],
                                                    HN[:, hsl(g, t)],
                                                    KD[t][:, g:g + 1])
                # hop 1: u <- kd2 o (A^T @ u)   (in place, via two psums)
                for g in range(G):
                    pss = []
                    for dh in range(2):
                        ps = pmm.tile([128, H1], dt, tag="psA")
                        for st in range(2):
                            ao = (g * 2 + st) * N + dh * 128
                            nc.tensor.matmul(ps[:], ADJ[:, ao:ao + 128],
                                             U[:, hsl(g, st)],
                                             start=(st == 0), stop=(st == 1))
                        pss.append(ps)
                    for dh in range(2):
                        nc.vector.tensor_scalar_mul(U[:, hsl(g, dh)], pss[dh][:],
                                                    KD2[dh][:, g:g + 1])
                # hop 2: h = kd o (A^T @ u)
                for g in range(G):
                    for dh in range(2):
                        ps = pmm.tile([128, H1], dt, tag="psA")
                        for st in range(2):
                            ao = (g * 2 + st) * N + dh * 128
                            nc.tensor.matmul(ps[:], ADJ[:, ao:ao + 128],
                                             U[:, hsl(g, st)],
                                             start=(st == 0), stop=(st == 1))
                        nc.vector.tensor_scalar_mul(HN[:, hsl(g, dh)], ps[:],
                                                    KD[dh][:, g:g + 1])

            def pool_readout(lvl):
                k = KS[lvl]
                # scores (columns), via fused mul+reduce on DVE
                for g in range(G):
                    for mt in range(2):
                        nc.vector.tensor_mul(U[:, hsl(g, mt)],
                                             HN[:, hsl(g, mt)], PWB[lvl][:])
                        nc.vector.tensor_reduce(SCC[mt][:, g:g + 1],
                                                U[:, hsl(g, mt)],
                                                axis=AX.X, op=OP.add)
                # score rows [16, 256]
                for mt in range(2):
                    pp = pt.tile([128, 128], dt, tag="psT")
                    nc.tensor.transpose(pp[0:16, :], SCC[mt][:], IDT[:])
                    nc.scalar.copy(SC[:, mt * 128:(mt + 1) * 128], pp[0:16, :])
                # mask inactive scores to -BIG
                nc.vector.tensor_scalar(AM16[:], KV[:], 1.0, BIG,
                                        op0=OP.subtract, op1=OP.mult)
                nc.vector.tensor_add(SCM[:], SC[:], AM16[:])
                # top-k mask via max8 + match_replace
                cur = SCM
                for it in range((k + 7) // 8):
                    nc.vector.max(TK8[:], cur[:])
                    rem = k - it * 8
                    if rem < 8:
                        nc.vector.memset(TK8[:, rem:8], MINV)
                    nc.vector.match_replace(WRK[:], TK8[:], cur[:], MINV)
                    cur = WRK
                nc.vector.tensor_sub(MSK[:], SCM[:], WRK[:])
                nc.vector.tensor_scalar_min(MSK[:], MSK[:], 1.0)
                # update keep state; pool scale ps = kv * tanh(score)
                nc.scalar.activation(TH[:], SC[:], AF.Tanh)
                nc.vector.tensor_mul(KV[:], KV[:], MSK[:])
                nc.vector.tensor_mul(PS[:], KV[:], TH[:])
                for mt in range(2):
                    for src, dsts in ((PS, PST), (KV, KVT)):
                        pp = pt.tile([128, 128], dt, tag="psT")
                        nc.tensor.transpose(pp[:, 0:16],
                                            src[0:16, mt * 128:(mt + 1) * 128],
                                            IDT[0:16, 0:16])
                        nc.vector.tensor_copy(dsts[mt][:], pp[:, 0:16])
                # h <- h * ps  (zeroes dropped rows, scales kept by tanh)
                for g in range(G):
                    for t in range(2):
                        nc.vector.tensor_scalar_mul(HN[:, hsl(g, t)],
                                                    HN[:, hsl(g, t)],
                                                    PST[t][:, g:g + 1])
                # transpose to feature-major HT
                i = 0
                for g in range(G):
                    for mt in range(2):
                        for ft in range(2):
                            pp = pt.tile([128, 128], dt, tag="psT")
                            o = (g * 2 + mt) * N + ft * 128
                            nc.tensor.transpose(pp[:], HN[:, o:o + 128], IDT[:])
                            dst = HT[ft][:, g * N + mt * 128:
                                          g * N + mt * 128 + 128]
                            if i % 2 == 0:
                                nc.scalar.copy(dst, pp[:])
                            else:
                                nc.vector.tensor_copy(dst, pp[:])
                            i += 1
                # readout: additive mask AMB = (kv-1)*BIG broadcast over parts
                nc.sync.dma_start(U[0:1, 0:GN], KV[:])
                for c in range(8):
                    pb = pmm.tile([128, 512], dt, tag="psA")
                    nc.tensor.matmul(pb[:], ONESR[0:1, :],
                                     U[0:1, c * 512:(c + 1) * 512],
                                     start=True, stop=True)
                    nc.vector.tensor_scalar(U[:, GN + c * 512:GN + (c + 1) * 512],
                                            pb[:], 1.0, BIG, op0=OP.subtract,
                                            op1=OP.mult)
                for ft in range(2):
                    nc.vector.tensor_add(U[:, 0:GN], HT[ft][:], U[:, GN:2 * GN])
                    nc.vector.tensor_reduce(
                        ZTL[:, ft * 16:(ft + 1) * 16],
                        U[:, 0:GN].rearrange("p (g n) -> p g n", g=G),
                        axis=AX.X, op=OP.max)
                    nc.vector.tensor_reduce(
                        ZTL[:, (2 + ft) * 16:(3 + ft) * 16],
                        HT[ft][:].rearrange("p (g n) -> p g n", g=G),
                        axis=AX.X, op=OP.add)
                nc.vector.tensor_scalar_mul(ZTL[:, 32:64], ZTL[:, 32:64],
                                            1.0 / k)
                nc.vector.tensor_add(ZACC[:], ZACC[:], ZTL[:])

            # ---- the network
            dense(1)
            for lvl in range(3):
                prop()
                pool_readout(lvl)
                if lvl < 2:
                    dense(lvl + 2)

            # ---- final MLP + log_softmax
            ps1 = pmm.tile([128, H1], dt, tag="psA")
            for kt in range(4):
                nc.tensor.matmul(ps1[0:16, :], ZACC[:, kt * 16:(kt + 1) * 16],
                                 W1S[:, kt * H1:(kt + 1) * H1],
                                 start=(kt == 0), stop=False)
            nc.tensor.matmul(ps1[0:16, :], ONESR[0:1, 0:16], B1R[:],
                             start=False, stop=True)
            nc.scalar.activation(Z1[:], ps1[0:16, :], AF.Relu)
            for kt in range(2):
                pp = pt.tile([128, 128], dt, tag="psT")
                nc.tensor.transpose(pp[:, 0:16],
                                    Z1[0:16, kt * 128:(kt + 1) * 128],
                                    IDT[0:16, 0:16])
                nc.scalar.copy(Z1T[:, kt * 16:(kt + 1) * 16], pp[:, 0:16])
            ps2 = pmm.tile([128, H2], dt, tag="psA")
            for kt in range(2):
                nc.tensor.matmul(ps2[0:16, :], Z1T[:, kt * 16:(kt + 1) * 16],
                                 W2S[:, kt * H2:(kt + 1) * H2],
                                 start=(kt == 0), stop=False)
            nc.tensor.matmul(ps2[0:16, :], ONESR[0:1, 0:16], B2R[:],
                             start=False, stop=True)
            nc.scalar.activation(Z2[:], ps2[0:16, :], AF.Relu)
            pp = pt.tile([128, 128], dt, tag="psT")
            nc.tensor.transpose(pp[:, 0:16], Z2[0:16, :], IDT[0:16, 0:16])
            nc.scalar.copy(Z2T[:], pp[:, 0:16])
            ps3 = pmm.tile([128, C], dt, tag="psA")
            nc.tensor.matmul(ps3[0:16, :], Z2T[:], W3S[:], start=True,
                             stop=False)
            nc.tensor.matmul(ps3[0:16, :], ONESR[0:1, 0:16], B3R[:],
                             start=False, stop=True)
            nc.vector.tensor_reduce(M16[:], ps3[0:16, :], axis=AX.X, op=OP.max)
            nc.vector.tensor_scalar_mul(NM16[:], M16[:], -1.0)
            nc.scalar.activation(ES[:], ps3[0:16, :], AF.Exp,
                                 bias=NM16[0:16, 0:1], scale=1.0)
            nc.vector.tensor_reduce(SE[:], ES[:], axis=AX.X, op=OP.add)
            nc.scalar.activation(LSE[:], SE[:], AF.Ln)
            nc.vector.tensor_scalar(OUTS[:], ps3[0:16, :], M16[0:16, 0:1],
                                    LSE[0:16, 0:1], op0=OP.subtract,
                                    op1=OP.subtract)
            nc.sync.dma_start(out_d[:], OUTS[:])

    nc.compile()
    return nc


def _get_nc():
    if "nc" not in _CACHE:
        _CACHE["nc"] = _build()
    return _CACHE["nc"]


def _host_prep(inputs):
    """Build per-core input maps (shared weights + per-core x/adj slices)."""
    x = np.ascontiguousarray(np.asarray(inputs["x"], np.float32))
    edges = np.asarray(inputs["edges"], np.int32)
    # dense adjacency counts + self loop: A[g, s, d] = #edges(s->d) + I
    src = edges[..., 0].astype(np.int64)
    dst = edges[..., 1].astype(np.int64)
    gidx = np.arange(B, dtype=np.int64)[:, None]
    flat = (gidx * N * N + src * N + dst).ravel()
    A = np.bincount(flat, minlength=B * N * N).astype(np.float32)
    A = A.reshape(B, N, N)
    A += np.eye(N, dtype=np.float32)[None]

    shared = {}
    for name, key in (("w12", "W12"), ("w22", "W22"), ("w32", "W32"),
                      ("w1", "W1"), ("w2", "W2"), ("w3", "W3")):
        shared[name] = np.ascontiguousarray(np.asarray(inputs[key], np.float32))
    for name, key, n in (("b12", "b12", H1), ("b22", "b22", H1),
                         ("b32", "b32", H1), ("b1", "b1", H1),
                         ("b2", "b2", H2), ("b3", "b3", C)):
        shared[name] = np.asarray(inputs[key], np.float32).reshape(1, n)
    for i, key in enumerate(("pw1", "pw2", "pw3")):
        pw = np.asarray(inputs[key], np.float32)
        pwn = pw / np.linalg.norm(pw)
        shared[f"pwb{i}"] = np.ascontiguousarray(
            np.broadcast_to(pwn[None, :], (128, H1)).astype(np.float32))

    in_maps = []
    for c in range(NCORES):
        m = dict(shared)
        m["x"] = np.ascontiguousarray(x[c * G * N:(c + 1) * G * N])
        m["adj"] = np.ascontiguousarray(A[c * G:(c + 1) * G])
        in_maps.append(m)
    return in_maps


def kernel(**inputs):
    from concourse.bass_utils import run_bass_kernel_spmd
    nc = _get_nc()
    in_maps = _host_prep(inputs)
    r = run_bass_kernel_spmd(nc, in_maps, core_ids=list(range(NCORES)))
    out = np.concatenate([r.results[c]["out"] for c in range(NCORES)], axis=0)
    return out.astype(np.float32)


def run_traced(inputs):
    """Like kernel() but with NTFF tracing; returns (out, BassKernelResults)."""
    import sys
    import types
    if "antenv.axon_hooks" not in sys.modules:
        hooks = types.ModuleType("antenv.axon_hooks")
        hooks._hook = None
        hooks.set_axon_ntff_profile_hook = lambda h: setattr(hooks, "_hook", h)
        hooks.get_axon_ntff_profile_hook = lambda: hooks._hook
        sys.modules["antenv.axon_hooks"] = hooks
        from trn_agent_boot.trn_boot import _ntff_profile_via_ctypes
        hooks.set_axon_ntff_profile_hook(
            _ntff_profile_via_ctypes("/opt/axon/libaxon_pjrt.so"))
    from concourse.bass_utils import run_bass_kernel_spmd
    nc = _get_nc()
    in_maps = _host_prep(inputs)
    r = run_bass_kernel_spmd(nc, in_maps, core_ids=list(range(NCORES)),
                             trace=True)
    out = np.concatenate([r.results[c]["out"] for c in range(NCORES)], axis=0)
    return out.astype(np.float32), r
